# revision 1
# baseline (speedup 1.0000x reference)
"""Masked multi-query-free attention for (B=8, S=2048, E=A=256), f32.

Sharding: data-parallel over batch B across the 8 NeuronCores (one batch
element per core, no collectives).

Per-core dataflow (all on-chip after the input DMAs):
  xT[E,S] -> qT,kT ([A,S], A on partitions; bias added on evacuation)
          -> v [S, A+2] (bias via K=1 ones-row matmul; cols A,A+1 hold 1.0
             so the PV matmul also produces the softmax denominator; the
             width is A+2 to satisfy fp32r/PE even-count rules)
  scores are computed TRANSPOSED: scT[sk_chunk=128p, sq] = kT.T @ qT
  attnT = exp(scT/16) * maskT  (no max subtraction needed: |scores|<~3)
  outP[sq=128p, A+2] += attnT_chunk.T @ v_chunk   (accumulate over sk)
  out = outP[:, :A] * (1 / outP[:, A])            (per-partition scale)

Matmul operands are fp16 (full-rate PE streaming, FWL weight loads, DVE 2x
elementwise); accumulation stays fp32 in PSUM, output is fp32. Junk warm-up
matmuls run during the DMA head so the PE HAM clock-gate opens early.
"""

import sys

sys.path.insert(0, "/opt/trn_rl_repo")

import numpy as np
import ml_dtypes

B, S, E, A = 8, 2048, 256, 256
N_CORES = 8

SQBLK = 512                 # Sq rows per outer block
N_SQBLK = S // SQBLK        # 4
SQSUB = 128                 # Sq rows per PV psum tile
N_SQSUB = SQBLK // SQSUB    # 4
SKCH = 128                  # Sk rows per score chunk (psum partitions)
N_SKCH = S // SKCH          # 16
GRP = 1                     # sk chunks per scores psum tile ([128, GRP*SQBLK])
N_GRP = N_SKCH // GRP
MTILE = 4                   # sk chunks per mask sbuf tile

SCALE = 1.0 / np.sqrt(np.float32(A))

import os as _os

MM_DT = _os.environ.get("KMMDT", "fp16")  # "fp16" | "f32r"


def _emit(nc, tc, ctx, tensors):
    import concourse.bass as bass
    import concourse.mybir as mybir

    f32 = mybir.dt.float32
    f32r = mybir.dt.float32r
    mdt = mybir.dt.float16 if MM_DT == "fp16" else f32r
    AF = mybir.ActivationFunctionType

    xT, maskT, Wq, bias_pack, Wk, Wv, row_pack, out = tensors

    def r(ap):  # matmul operand dtype fixup (f32 -> f32r fast mode)
        if MM_DT != "fp16" and ap.dtype == f32:
            return ap.bitcast(f32r)
        return ap

    consts = ctx.enter_context(tc.tile_pool(name="consts", bufs=1))
    big = ctx.enter_context(tc.tile_pool(name="big", bufs=1))
    mpool = ctx.enter_context(tc.tile_pool(name="mask", bufs=16))
    epool = ctx.enter_context(tc.tile_pool(name="exp", bufs=6))
    apool = ctx.enter_context(tc.tile_pool(name="attn", bufs=6))
    opool = ctx.enter_context(tc.tile_pool(name="outsb", bufs=6))
    spool = ctx.enter_context(tc.tile_pool(name="small", bufs=8))
    ps_sc = ctx.enter_context(tc.tile_pool(name="ps_sc", bufs=4, space="PSUM"))
    ps_sm = ctx.enter_context(tc.tile_pool(name="ps_sm", bufs=4, space="PSUM"))

    # ---- HAM warm-up: junk matmuls during the input-DMA head so the PE
    # clock gate is already at 8/8 when real work arrives ----
    warm_sb = consts.tile([128, 512], f32, tag="warm_sb")
    nc.vector.memset(warm_sb, 1.0)
    warm_ps = ps_sc.tile([128, GRP * SQBLK], f32, name="warm_ps", tag="sc")
    for _ in range(4):
        nc.tensor.matmul(
            warm_ps[:, :512], lhsT=warm_sb[:, :128], rhs=warm_sb, start=True, stop=True
        )

    # ---- inputs: spread across the three DGE rings; weights first (small),
    # then x split per Sq-block so projections can start on partial data ----
    Wq_sb, Wk_sb, Wv_sb = [], [], []
    for e in range(2):
        wq = consts.tile([128, A], mdt, tag=f"wq{e}")
        nc.sync.dma_start(out=wq, in_=Wq[e])
        Wq_sb.append(wq)
    for e in range(2):
        wk = consts.tile([128, A], mdt, tag=f"wk{e}")
        nc.gpsimd.dma_start(out=wk, in_=Wk[e])
        Wk_sb.append(wk)
        wv = consts.tile([128, A + 2], mdt, tag=f"wv{e}")
        nc.gpsimd.dma_start(out=wv, in_=Wv[e])
        Wv_sb.append(wv)
    bias_sb = consts.tile([128, 4], f32, tag="bias_pack")
    nc.sync.dma_start(out=bias_sb, in_=bias_pack)
    bq_sb = [bias_sb[:, 0:1], bias_sb[:, 1:2]]
    bk_sb = [bias_sb[:, 2:3], bias_sb[:, 3:4]]
    row_sb = consts.tile([1, A + 2 + 128], mdt, tag="row_pack")
    nc.sync.dma_start(out=row_sb, in_=row_pack)
    bv_sb = row_sb[:, : A + 2]
    ones_sb = row_sb[:, A + 2 :]

    # xT as per-Sq-block tiles, (e0,e1) pairs interleaved so early j-blocks
    # land first: j0/j1 via the gpsimd ring, j2/j3 via sync
    xT_sb = [[None] * N_SQBLK, [None] * N_SQBLK]
    for j in range(N_SQBLK):
        for e in range(2):
            t = big.tile([128, SQBLK], mdt, name=f"xt{e}_{j}", tag=f"xT{e}_{j}")
            (nc.gpsimd if j < 2 else nc.sync).dma_start(
                out=t, in_=xT[e][:, bass.ts(j, SQBLK)]
            )
            xT_sb[e][j] = t

    # ---- projections ----
    # qT/kT: [A-chunk=128p, S]; psum tile per (a, Sq512), accumulate E chunks.
    qT_sb, kT_sb = [], []
    for a in range(2):
        qt = big.tile([128, S], mdt, tag=f"qT{a}")
        kt = big.tile([128, S], mdt, tag=f"kT{a}")
        qT_sb.append(qt)
        kT_sb.append(kt)
    # Interleaved projection steps: each step emits one qk psum-pair (PE,
    # evacuated on DVE) plus two v chunks (PE, evacuated on ACT). The v
    # matmuls keep the same moving operand across consecutive MMs so their
    # weight loads pipeline; the PE stays dense while DVE drains qk psums.
    v_sb = [None] * N_SKCH
    qk_steps = [
        (jp, a, wi)
        for jp in ((0, 1), (2, 3))
        for a in range(2)
        for wi in range(2)
    ]
    for s, (jp, a, wi) in enumerate(qk_steps):
        W_sb, b_sb, dst = (
            (Wq_sb, bq_sb[a], qT_sb[a]),
            (Wk_sb, bk_sb[a], kT_sb[a]),
        )[wi]
        pss = [
            ps_sm.tile([128, 512], f32, name=f"pp{s}_{j}", tag="ps") for j in jp
        ]
        for e in range(2):
            for i, j in enumerate(jp):
                nc.tensor.matmul(
                    pss[i],
                    lhsT=r(W_sb[e][:, bass.ts(a, 128)]),
                    rhs=r(xT_sb[e][j]),
                    start=(e == 0),
                    stop=(e == 1),
                )
        cpair = (2 * s, 2 * s + 1)
        vps = [
            ps_sm.tile([128, 512], f32, name=f"vp{c}", tag="ps") for c in cpair
        ]
        for e in range(2):
            for i, c in enumerate(cpair):
                nc.tensor.matmul(
                    vps[i][:, : A + 2],
                    lhsT=r(xT_sb[e][c // 4][:, bass.ts(c % 4, 128)]),
                    rhs=r(Wv_sb[e]),
                    start=(e == 0),
                    stop=False,
                )
        for i, c in enumerate(cpair):
            nc.tensor.matmul(
                vps[i][:, : A + 2],
                lhsT=r(ones_sb),
                rhs=r(bv_sb),
                start=False,
                stop=True,
            )
        for i, j in enumerate(jp):
            nc.vector.tensor_scalar_add(dst[:, bass.ts(j, SQBLK)], pss[i], b_sb)
        for i, c in enumerate(cpair):
            vt = big.tile([128, A + 2], mdt, tag=f"v{c}", name=f"v{c}")
            nc.scalar.copy(vt, vps[i][:, : A + 2])
            v_sb[c] = vt

    # ---- attention ----
    mask_tiles = {}
    for j in range(N_SQBLK):
        for t in range(N_SKCH // MTILE):
            mt = mpool.tile(
                [128, MTILE, SQBLK], mybir.dt.float16 if MM_DT == "fp16" else mybir.dt.bfloat16, name=f"mask{j}_{t}", tag="mask"
            )
            nc.gpsimd.dma_start(out=mt, in_=maskT[j][:, bass.ts(t, MTILE), :])
            mask_tiles[(j, t)] = mt

    for j in range(N_SQBLK):
        js = bass.ts(j, SQBLK)
        mask_sb = [mask_tiles[(j, t)] for t in range(N_SKCH // MTILE)]

        out_ps = [
            ps_sm.tile([128, 512], f32, name=f"out_ps{j}_{s}", tag="ps")
            for s in range(N_SQSUB)
        ]

        for g in range(N_GRP):
            sc = ps_sc.tile([128, GRP * SQBLK], f32)
            for c in range(GRP):
                ch = g * GRP + c
                for a in range(2):
                    nc.tensor.matmul(
                        sc[:, bass.ts(c, SQBLK)],
                        lhsT=r(kT_sb[a][:, bass.ts(ch, 128)]),
                        rhs=r(qT_sb[a][:, js]),
                        start=(a == 0),
                        stop=(a == 1),
                    )
            ex = epool.tile([128, GRP * SQBLK], mdt if MM_DT == "fp16" else f32)
            nc.scalar.activation(ex, sc, AF.Exp, bias=0.0, scale=float(SCALE))
            at = apool.tile([128, GRP, SQBLK], mdt)
            mslice = mask_sb[(g * GRP) // MTILE][:, bass.ds((g * GRP) % MTILE, GRP), :]
            nc.vector.tensor_mul(at, ex.rearrange("p (c s) -> p c s", c=GRP), mslice)
            for c in range(GRP):
                ch = g * GRP + c
                for sq in range(N_SQSUB):
                    nc.tensor.matmul(
                        out_ps[sq][:, : A + 2],
                        lhsT=r(at[:, c, bass.ts(sq, SQSUB)]),
                        rhs=r(v_sb[ch]),
                        start=(ch == 0),
                        stop=(ch == N_SKCH - 1),
                    )

        for sq in range(N_SQSUB):
            rec = spool.tile([128, 1], f32)
            nc.vector.reciprocal(rec, out_ps[sq][:, A : A + 1])
            ob = opool.tile([128, A], f32)
            nc.scalar.mul(ob, out_ps[sq][:, :A], rec)
            nc.sync.dma_start(
                out=out[j * SQBLK + sq * SQSUB : j * SQBLK + (sq + 1) * SQSUB, :],
                in_=ob,
            )


def build_nc():
    from contextlib import ExitStack

    import concourse.bacc as bacc
    import concourse.tile as tile
    import concourse.mybir as mybir

    f32 = mybir.dt.float32
    f32r = mybir.dt.float32r
    mdt = mybir.dt.float16 if MM_DT == "fp16" else f32r
    maskdt = mybir.dt.float16 if MM_DT == "fp16" else mybir.dt.bfloat16

    nc = bacc.Bacc("TRN2", target_bir_lowering=False, debug=False)
    xT = nc.dram_tensor("xT", [2, 128, S], mdt, kind="ExternalInput").ap()
    maskT = nc.dram_tensor(
        "maskT", [N_SQBLK, 128, N_SKCH, SQBLK], maskdt, kind="ExternalInput"
    ).ap()
    Wq = nc.dram_tensor("Wq", [2, 128, A], mdt, kind="ExternalInput").ap()
    Wk = nc.dram_tensor("Wk", [2, 128, A], mdt, kind="ExternalInput").ap()
    Wv = nc.dram_tensor("Wv", [2, 128, A + 2], mdt, kind="ExternalInput").ap()
    bias_pack = nc.dram_tensor("bias_pack", [128, 4], f32, kind="ExternalInput").ap()
    row_pack = nc.dram_tensor(
        "row_pack", [1, A + 2 + 128], mdt, kind="ExternalInput"
    ).ap()
    out = nc.dram_tensor("out", [S, A], f32, kind="ExternalOutput").ap()

    tensors = (xT, maskT, Wq, bias_pack, Wk, Wv, row_pack, out)
    with tile.TileContext(nc) as tc:
        with ExitStack() as ctx:
            _emit(nc, tc, ctx, tensors)
    nc.compile()
    return nc


def pack_inputs(x, mask, Wq, bq, Wk, bk, Wv, bv):
    """Host-side packing: per-core input maps (core c <- batch c)."""
    hdt = np.float16 if MM_DT == "fp16" else np.float32
    hmaskdt = np.float16 if MM_DT == "fp16" else ml_dtypes.bfloat16
    x = np.asarray(x, dtype=np.float32)
    mask = np.asarray(mask)
    # maskT[b, j, p, c, s] = mask[b, j*512+s, c*128+p], as {0.0, 1.0}
    from concurrent.futures import ThreadPoolExecutor

    def _pack_mask(b):
        return np.ascontiguousarray(
            mask[b]
            .transpose(1, 0)
            .reshape(N_SKCH, 128, N_SQBLK, SQBLK)
            .transpose(2, 1, 0, 3)
            .astype(hmaskdt)
        )

    with ThreadPoolExecutor(max_workers=8) as tp:
        mt = list(tp.map(_pack_mask, range(B)))
    Wq = np.ascontiguousarray(np.asarray(Wq, hdt)).reshape(2, 128, A)
    Wk = np.ascontiguousarray(np.asarray(Wk, hdt)).reshape(2, 128, A)
    Wv = np.ascontiguousarray(
        np.concatenate(
            [np.asarray(Wv, hdt), np.zeros((E, 2), hdt)], axis=1
        ).reshape(2, 128, A + 2)
    )
    bq = np.asarray(bq, np.float32).reshape(2, 128)
    bk = np.asarray(bk, np.float32).reshape(2, 128)
    bias_pack = np.ascontiguousarray(
        np.stack([bq[0], bq[1], bk[0], bk[1]], axis=1)
    )
    row_pack = np.concatenate(
        [
            np.asarray(bv, hdt).reshape(-1),
            np.ones(2, hdt),
            np.ones(128, hdt),
        ]
    ).reshape(1, A + 2 + 128)
    in_maps = []
    for b in range(N_CORES):
        xb = np.ascontiguousarray(x[b].T.astype(hdt)).reshape(2, 128, S)
        in_maps.append(
            {
                "xT": xb,
                "maskT": mt[b],
                "Wq": Wq,
                "Wk": Wk,
                "Wv": Wv,
                "bias_pack": bias_pack,
                "row_pack": row_pack,
            }
        )
    return in_maps


_NC_CACHE = None


def _get_nc():
    global _NC_CACHE
    if _NC_CACHE is None:
        _NC_CACHE = build_nc()
    return _NC_CACHE


def kernel(x, mask, Wq, bq, Wk, bk, Wv, bv):
    from concourse.bass_utils import run_bass_kernel_spmd

    in_maps = pack_inputs(x, mask, Wq, bq, Wk, bk, Wv, bv)
    nc = _get_nc()
    res = run_bass_kernel_spmd(nc, in_maps, core_ids=list(range(N_CORES)))
    out = np.stack([res.results[c]["out"] for c in range(N_CORES)], axis=0)
    return out.astype(np.float32)


if __name__ == "__main__":
    nc = build_nc()
    n = sum(len(bb.instructions) for bb in nc.main_func.blocks)
    print("built ok; instructions:", n)



# revision 3
# speedup vs baseline: 1.0088x; 1.0088x over previous
"""Masked attention for (B=8, S=2048, E=A=256), f32 in/out.

Sharding: data-parallel over batch B across the 8 NeuronCores (one batch
element per core, no collectives).

Per-core dataflow (all on-chip after the input DMAs):
  xT[E,S] -> qT8,kT8 ([128, 2, S] fp8e4, a-dim split in 2 k-tiles; bias
             added during the DVE psum evacuation that also quantizes)
          -> v [S, A+2] fp16 (bias via K=1 ones-row matmul; cols A,A+1 are
             1.0 so the PV matmul also produces the softmax denominator)
  scores computed TRANSPOSED via fp8 DoubleRow matmuls (K=256 in one
  pass, 2x PE rate): scT[sk 64-sub, sq 256-sub] tiles into a 2-chunk
  [128, 1024] psum tile (2 banks).
  One exp per 2-chunk tile on ACT (scale=1/16, psum->fp16 sbuf).
  One mask multiply per tile on DVE (fp16 2x mode).
  outP[sq=128p, A+2] += attnT_chunk.T @ v_chunk  (fp16 PV, accumulate sk)
  out = outP[:, :A] * (1 / outP[:, A])           (DVE recip + per-part mul)

Input DMAs: per-j 2.1MB mask transfers split across the scalar HWDGE ring
(j0, j2) and the gpsimd SWDGE ring (j1, j3); x/weights/out on sync HWDGE.
Output is fp16 [j, p, sq, a], unpacked+cast on host.
"""

import sys

sys.path.insert(0, "/opt/trn_rl_repo")

import numpy as np

B, S, E, A = 8, 2048, 256, 256
N_CORES = 8

SQBLK = 512                 # Sq rows per outer block
N_SQBLK = S // SQBLK        # 4
SQSUB = 128                 # Sq rows per PV psum tile
N_SQSUB = SQBLK // SQSUB    # 4
SKCH = 128                  # Sk rows per score chunk
N_SKCH = S // SKCH          # 16
GRP = 2                     # sk chunks per scores psum tile ([128, GRP*512])
N_GRP = N_SKCH // GRP       # 8

SCALE = 1.0 / np.sqrt(np.float32(A))


def _emit(nc, tc, ctx, tensors):
    import concourse.bass as bass
    import concourse.mybir as mybir

    f32 = mybir.dt.float32
    f16 = mybir.dt.float16
    f8 = mybir.dt.float8e4
    AF = mybir.ActivationFunctionType
    DR = mybir.MatmulPerfMode.DoubleRow

    xT, maskT, Wq, bias_pack, Wk, Wv, row_pack, out = tensors

    consts = ctx.enter_context(tc.tile_pool(name="consts", bufs=1))
    big = ctx.enter_context(tc.tile_pool(name="big", bufs=1))
    mpool = ctx.enter_context(tc.tile_pool(name="mask", bufs=4))
    epool = ctx.enter_context(tc.tile_pool(name="exp", bufs=4))
    apool = ctx.enter_context(tc.tile_pool(name="attn", bufs=4))
    opool = ctx.enter_context(tc.tile_pool(name="outsb", bufs=2))
    spool = ctx.enter_context(tc.tile_pool(name="small", bufs=8))
    ps_sc = ctx.enter_context(tc.tile_pool(name="ps_sc", bufs=2, space="PSUM"))
    ps_sm = ctx.enter_context(tc.tile_pool(name="ps_sm", bufs=4, space="PSUM"))

    # ---- ACT exp-table preload + PE HAM warm-up during the DMA head ----
    warm_sb = consts.tile([128, 512], f16, tag="warm_sb")
    nc.vector.memset(warm_sb, 1.0)
    warm_ex = consts.tile([128, 1], f16, tag="warm_ex")
    nc.scalar.activation(warm_ex, warm_sb[:, 0:1], AF.Exp, bias=0.0, scale=0.001)
    warm_ps = ps_sc.tile([128, GRP * SQBLK], f32, name="warm_ps", tag="sc")
    for _ in range(4):
        nc.tensor.matmul(
            warm_ps[:, :512], lhsT=warm_sb[:, :128], rhs=warm_sb, start=True, stop=True
        )

    # ---- input DMAs ----
    # sync HWDGE: Wq, x (j-order), bias/row packs; later the out stores.
    Wq_sb, Wk_sb, Wv_sb = [], [], []
    for e in range(2):
        wq = consts.tile([128, A], f16, tag=f"wq{e}")
        nc.sync.dma_start(out=wq, in_=Wq[e])
        Wq_sb.append(wq)
    # gpsimd SWDGE: Wk, Wv first (small), then mask j1, j3.
    for e in range(2):
        wk = consts.tile([128, A], f16, tag=f"wk{e}")
        nc.gpsimd.dma_start(out=wk, in_=Wk[e])
        Wk_sb.append(wk)
        wv = consts.tile([128, A + 2], f16, tag=f"wv{e}")
        nc.gpsimd.dma_start(out=wv, in_=Wv[e])
        Wv_sb.append(wv)

    xT_sb = [[None] * N_SQBLK, [None] * N_SQBLK]
    for j in range(N_SQBLK):
        for e in range(2):
            t = big.tile([128, SQBLK], f16, name=f"xt{e}_{j}", tag=f"xT{e}_{j}")
            nc.sync.dma_start(out=t, in_=xT[e][:, bass.ts(j, SQBLK)])
            xT_sb[e][j] = t
    bias_sb = consts.tile([128, 4], f32, tag="bias_pack")
    nc.sync.dma_start(out=bias_sb, in_=bias_pack)
    bq_sb = [bias_sb[:, 0:1], bias_sb[:, 1:2]]
    bk_sb = [bias_sb[:, 2:3], bias_sb[:, 3:4]]
    row_sb = consts.tile([1, A + 2 + 128], f16, tag="row_pack")
    nc.sync.dma_start(out=row_sb, in_=row_pack)
    bv_sb = row_sb[:, : A + 2]
    ones_sb = row_sb[:, A + 2 :]

    # masks: one 2.1MB DMA per j-block; j0/j2 on the scalar HWDGE ring,
    # j1/j3 on the gpsimd SWDGE ring (after the small weight loads).
    mask_sb = [None] * N_SQBLK
    for j in (0, 1, 2, 3):
        mt = mpool.tile([128, N_SKCH, SQBLK], f16, name=f"mask{j}", tag="mask")
        (nc.scalar if j % 2 == 0 else nc.gpsimd).dma_start(out=mt, in_=maskT[j])
        mask_sb[j] = mt

    # ---- projections ----
    # qT8/kT8: [128, 2, S] fp8e4; [p, t, s] = (q|k)[s, t*128+p].
    qT8 = big.tile([128, 2, S], f8, tag="qT8")
    kT8 = big.tile([128, 2, S], f8, tag="kT8")
    # Interleaved steps: each emits one qk psum-pair (PE; evacuated+quantized
    # on DVE) plus two v chunks (PE; evacuated on ACT). v matmuls keep the
    # same moving operand across consecutive MMs so weight loads pipeline.
    v_sb = [None] * N_SKCH
    qk_steps = [
        (jp, a, wi)
        for jp in ((0, 1), (2, 3))
        for a in range(2)
        for wi in range(2)
    ]
    for s, (jp, a, wi) in enumerate(qk_steps):
        W_sb, b_sb, dst = (
            (Wq_sb, bq_sb[a], qT8),
            (Wk_sb, bk_sb[a], kT8),
        )[wi]
        pss = [
            ps_sm.tile([128, 512], f32, name=f"pp{s}_{j}", tag="ps") for j in jp
        ]
        for e in range(2):
            for i, j in enumerate(jp):
                nc.tensor.matmul(
                    pss[i],
                    lhsT=W_sb[e][:, bass.ts(a, 128)],
                    rhs=xT_sb[e][j],
                    start=(e == 0),
                    stop=(e == 1),
                )
        cpair = (2 * s, 2 * s + 1)
        vps = [
            ps_sm.tile([128, 512], f32, name=f"vp{c}", tag="ps") for c in cpair
        ]
        for e in range(2):
            for i, c in enumerate(cpair):
                nc.tensor.matmul(
                    vps[i][:, : A + 2],
                    lhsT=xT_sb[e][c // 4][:, bass.ts(c % 4, 128)],
                    rhs=Wv_sb[e],
                    start=(e == 0),
                    stop=False,
                )
        for i, c in enumerate(cpair):
            nc.tensor.matmul(
                vps[i][:, : A + 2],
                lhsT=ones_sb,
                rhs=bv_sb,
                start=False,
                stop=True,
            )
        for i, j in enumerate(jp):
            nc.vector.tensor_scalar_add(
                dst[:, a, bass.ts(j, SQBLK)], pss[i], b_sb
            )
        for i, c in enumerate(cpair):
            vt = big.tile([128, A + 2], f16, tag=f"v{c}", name=f"v{c}")
            nc.scalar.copy(vt, vps[i][:, : A + 2])
            v_sb[c] = vt

    # ---- attention ----
    for j in range(N_SQBLK):
        out_ps = [
            ps_sm.tile([128, 512], f32, name=f"out_ps{j}_{s}", tag="ps")
            for s in range(N_SQSUB)
        ]

        for g in range(N_GRP):
            sc = ps_sc.tile([128, GRP * SQBLK], f32, tag="sc")
            for c in range(GRP):
                ch = g * GRP + c
                for sqh in range(2):  # sq 256-col halves (DR moving limit)
                    nc.tensor.matmul(
                        sc[:, c * SQBLK + sqh * 256 : c * SQBLK + (sqh + 1) * 256],
                        lhsT=kT8[:, :, ch * 128 : (ch + 1) * 128],
                        rhs=qT8[:, :, j * SQBLK + sqh * 256 : j * SQBLK + (sqh + 1) * 256],
                        start=True,
                        stop=True,
                        perf_mode=DR,
                    )
            ex = epool.tile([128, GRP * SQBLK], f16, tag="ex")
            nc.scalar.activation(ex, sc, AF.Exp, bias=0.0, scale=float(SCALE))
            at = apool.tile([128, GRP, SQBLK], f16, tag="at")
            nc.vector.tensor_mul(
                at,
                ex.rearrange("p (c s) -> p c s", c=GRP),
                mask_sb[j][:, g * GRP : (g + 1) * GRP, :],
            )
            for c in range(GRP):
                ch = g * GRP + c
                for sq in range(N_SQSUB):
                    nc.tensor.matmul(
                        out_ps[sq][:, : A + 2],
                        lhsT=at[:, c, bass.ts(sq, SQSUB)],
                        rhs=v_sb[ch],
                        start=(ch == 0),
                        stop=(ch == N_SKCH - 1),
                    )

        ob = opool.tile([128, N_SQSUB, A], f16, tag="ob")
        for sq in range(N_SQSUB):
            rec = spool.tile([128, 1], f32, tag="rec")
            nc.vector.reciprocal(rec, out_ps[sq][:, A : A + 1])
            nc.vector.tensor_scalar_mul(ob[:, sq, :], out_ps[sq][:, :A], rec)
        nc.sync.dma_start(out=out[j], in_=ob)


def build_nc():
    from contextlib import ExitStack

    import concourse.bacc as bacc
    import concourse.tile as tile
    import concourse.mybir as mybir

    f32 = mybir.dt.float32
    f16 = mybir.dt.float16

    nc = bacc.Bacc("TRN2", target_bir_lowering=False, debug=False)
    xT = nc.dram_tensor("xT", [2, 128, S], f16, kind="ExternalInput").ap()
    maskT = nc.dram_tensor(
        "maskT", [N_SQBLK, 128, N_SKCH, SQBLK], f16, kind="ExternalInput"
    ).ap()
    Wq = nc.dram_tensor("Wq", [2, 128, A], f16, kind="ExternalInput").ap()
    Wk = nc.dram_tensor("Wk", [2, 128, A], f16, kind="ExternalInput").ap()
    Wv = nc.dram_tensor("Wv", [2, 128, A + 2], f16, kind="ExternalInput").ap()
    bias_pack = nc.dram_tensor("bias_pack", [128, 4], f32, kind="ExternalInput").ap()
    row_pack = nc.dram_tensor(
        "row_pack", [1, A + 2 + 128], f16, kind="ExternalInput"
    ).ap()
    # out[j, p, q, a] = attention_out[j*512 + q*128 + p, a], fp16
    out = nc.dram_tensor(
        "out", [N_SQBLK, 128, N_SQSUB, A], f16, kind="ExternalOutput"
    ).ap()

    tensors = (xT, maskT, Wq, bias_pack, Wk, Wv, row_pack, out)
    with tile.TileContext(nc) as tc:
        with ExitStack() as ctx:
            _emit(nc, tc, ctx, tensors)
    nc.compile()
    return nc


def pack_inputs(x, mask, Wq, bq, Wk, bk, Wv, bv):
    """Host-side packing: per-core input maps (core c <- batch c)."""
    hdt = np.float16
    x = np.asarray(x, dtype=np.float32)
    mask = np.asarray(mask)
    # maskT[b, j, p, c, s] = mask[b, j*512+s, c*128+p], as {0.0, 1.0}
    from concurrent.futures import ThreadPoolExecutor

    def _pack_mask(b):
        return np.ascontiguousarray(
            mask[b]
            .transpose(1, 0)
            .reshape(N_SKCH, 128, N_SQBLK, SQBLK)
            .transpose(2, 1, 0, 3)
            .astype(hdt)
        )

    with ThreadPoolExecutor(max_workers=8) as tp:
        mt = list(tp.map(_pack_mask, range(B)))
    Wq = np.ascontiguousarray(np.asarray(Wq, hdt)).reshape(2, 128, A)
    Wk = np.ascontiguousarray(np.asarray(Wk, hdt)).reshape(2, 128, A)
    Wv = np.ascontiguousarray(
        np.concatenate(
            [np.asarray(Wv, hdt), np.zeros((E, 2), hdt)], axis=1
        ).reshape(2, 128, A + 2)
    )
    bq = np.asarray(bq, np.float32).reshape(2, 128)
    bk = np.asarray(bk, np.float32).reshape(2, 128)
    bias_pack = np.ascontiguousarray(
        np.stack([bq[0], bq[1], bk[0], bk[1]], axis=1)
    )
    row_pack = np.concatenate(
        [
            np.asarray(bv, hdt).reshape(-1),
            np.ones(2, hdt),
            np.ones(128, hdt),
        ]
    ).reshape(1, A + 2 + 128)
    in_maps = []
    for b in range(N_CORES):
        xb = np.ascontiguousarray(x[b].T.astype(hdt)).reshape(2, 128, S)
        in_maps.append(
            {
                "xT": xb,
                "maskT": mt[b],
                "Wq": Wq,
                "Wk": Wk,
                "Wv": Wv,
                "bias_pack": bias_pack,
                "row_pack": row_pack,
            }
        )
    return in_maps


_NC_CACHE = None


def _get_nc():
    global _NC_CACHE
    if _NC_CACHE is None:
        _NC_CACHE = build_nc()
    return _NC_CACHE


def kernel(x, mask, Wq, bq, Wk, bk, Wv, bv):
    from concourse.bass_utils import run_bass_kernel_spmd

    in_maps = pack_inputs(x, mask, Wq, bq, Wk, bk, Wv, bv)
    nc = _get_nc()
    res = run_bass_kernel_spmd(nc, in_maps, core_ids=list(range(N_CORES)))
    # out[j, p, q, a] -> [j*512 + q*128 + p, a]
    outs = []
    for c in range(N_CORES):
        o = np.asarray(res.results[c]["out"])
        outs.append(o.transpose(0, 2, 1, 3).reshape(S, A))
    return np.stack(outs, axis=0).astype(np.float32)


if __name__ == "__main__":
    nc = build_nc()
    n = sum(len(bb.instructions) for bb in nc.main_func.blocks)
    print("built ok; instructions:", n)


# revision 4
# speedup vs baseline: 1.0563x; 1.0471x over previous
"""Masked attention for (B=8, S=2048, E=A=256), f32 in/out.

Sharding: data-parallel over batch B across the 8 NeuronCores (one batch
element per core, no collectives).

Per-core dataflow (all on-chip after the input DMAs):
  xT[E,S] -> qT8,kT8 ([128, 2, S] fp8e4, a-dim split in 2 k-tiles; bias
             added during the DVE psum evacuation that also quantizes)
          -> v [S, A+2] fp16 (bias via K=1 ones-row matmul; cols A,A+1 are
             1.0 so the PV matmul also produces the softmax denominator)
  scores computed TRANSPOSED via fp8 DoubleRow matmuls (full K=256 in one
  matmul, 2x PE rate): scT[sk chunk=128p, sq 256-sub] into a 2-chunk
  [128, 1024] psum tile (2 banks, double buffered).
  One exp per 2-chunk tile on ACT (scale=1/16, psum->fp16 sbuf); mask
  multiply per single chunk on DVE (fp16 2x mode) to cut the PV dep chain.
  PV is emitted TWO groups behind scores so the exp+mask latency hides
  under ~1.8us of independent PE work.
  outP[sq=128p, A+2] += attnT_chunk.T @ v_chunk  (fp16 PV, accumulate sk)
  out = outP[:, :A] * (1 / outP[:, A])  (DVE recip; per-partition muls
  split DVE/ACT to shorten the j-boundary bubble)

Input DMAs (no SWDGE): sync HWDGE carries one packed weight tensor, two
whole xT[e] transfers, masks j1/j3, and the per-j fp16 output stores;
scalar HWDGE carries masks j0/j2.
"""

import sys

sys.path.insert(0, "/opt/trn_rl_repo")

import numpy as np

B, S, E, A = 8, 2048, 256, 256
N_CORES = 8

SQBLK = 512                 # Sq rows per outer block
N_SQBLK = S // SQBLK        # 4
SQSUB = 128                 # Sq rows per PV psum tile
N_SQSUB = SQBLK // SQSUB    # 4
SKCH = 128                  # Sk rows per score chunk
N_SKCH = S // SKCH          # 16
GRP = 2                     # sk chunks per scores psum tile ([128, GRP*512])
N_GRP = N_SKCH // GRP       # 8
PVLAG = 2                   # groups PV trails scores in the PE stream

SCALE = 1.0 / np.sqrt(np.float32(A))

# wpack column layout (fp16): Wq e0|e1, Wk e0|e1, Wv e0|e1 (A+2 wide),
# then on partition 0 only: bv row (A+2) + ones row (128)
WQ_OFF = 0
WK_OFF = WQ_OFF + 2 * A
WV_OFF = WK_OFF + 2 * A
ROW_OFF = WV_OFF + 2 * (A + 2)
WPACK_F = ROW_OFF + (A + 2) + 128


def _emit(nc, tc, ctx, tensors):
    import concourse.bass as bass
    import concourse.mybir as mybir

    f32 = mybir.dt.float32
    f16 = mybir.dt.float16
    f8 = mybir.dt.float8e4
    AF = mybir.ActivationFunctionType
    DR = mybir.MatmulPerfMode.DoubleRow

    xT, maskT, wpack, bias_pack, out = tensors

    consts = ctx.enter_context(tc.tile_pool(name="consts", bufs=1))
    big = ctx.enter_context(tc.tile_pool(name="big", bufs=1))
    mpool = ctx.enter_context(tc.tile_pool(name="mask", bufs=4))
    epool = ctx.enter_context(tc.tile_pool(name="exp", bufs=4))
    apool = ctx.enter_context(tc.tile_pool(name="attn", bufs=6))
    opool = ctx.enter_context(tc.tile_pool(name="outsb", bufs=2))
    spool = ctx.enter_context(tc.tile_pool(name="small", bufs=8))
    psum = ctx.enter_context(tc.tile_pool(name="psum", bufs=1, space="PSUM"))

    # ---- ACT exp-table preload + PE HAM warm-up during the DMA head ----
    warm_sb = consts.tile([128, 512], f16, tag="warm_sb")
    nc.vector.memset(warm_sb, 1.0)
    warm_ex = consts.tile([128, 1], f16, tag="warm_ex")
    nc.scalar.activation(warm_ex, warm_sb[:, 0:1], AF.Exp, bias=0.0, scale=0.001)
    warm_ps = psum.tile([128, GRP * SQBLK], f32, name="warm_ps", tag="sc", bufs=2)
    for _ in range(4):
        nc.tensor.matmul(
            warm_ps[:, :512], lhsT=warm_sb[:, :128], rhs=warm_sb, start=True, stop=True
        )

    # ---- input DMAs ----
    wp = consts.tile([128, WPACK_F], f16, tag="wpack")
    nc.sync.dma_start(out=wp, in_=wpack)
    bias_sb = consts.tile([128, 4], f32, tag="bias_pack")
    nc.sync.dma_start(out=bias_sb, in_=bias_pack)
    Wq_sb = [wp[:, WQ_OFF + e * A : WQ_OFF + (e + 1) * A] for e in range(2)]
    Wk_sb = [wp[:, WK_OFF + e * A : WK_OFF + (e + 1) * A] for e in range(2)]
    Wv_sb = [
        wp[:, WV_OFF + e * (A + 2) : WV_OFF + (e + 1) * (A + 2)] for e in range(2)
    ]
    bq_sb = [bias_sb[:, 0:1], bias_sb[:, 1:2]]
    bk_sb = [bias_sb[:, 2:3], bias_sb[:, 3:4]]
    bv_sb = wp[0:1, ROW_OFF : ROW_OFF + A + 2]
    ones_sb = wp[0:1, ROW_OFF + A + 2 : ROW_OFF + A + 2 + 128]

    xT_sb = []
    for e in range(2):
        t = big.tile([128, S], f16, name=f"xt{e}", tag=f"xT{e}")
        nc.sync.dma_start(out=t, in_=xT[e])
        xT_sb.append(t)

    # masks: one 2.1MB DMA per j-block; j0/j2 on the scalar HWDGE ring,
    # j1/j3 on the sync ring (queued behind the small input transfers).
    mask_sb = [None] * N_SQBLK
    for j in (0, 2, 1, 3):
        mt = mpool.tile([128, N_SKCH, SQBLK], f16, name=f"mask{j}", tag="mask")
        (nc.scalar if j % 2 == 0 else nc.sync).dma_start(out=mt, in_=maskT[j])
        mask_sb[j] = mt

    # ---- projections ----
    # qT8/kT8: [128, 2, S] fp8e4; [p, t, s] = (q|k)[s, t*128+p].
    qT8 = big.tile([128, 2, S], f8, tag="qT8")
    kT8 = big.tile([128, 2, S], f8, tag="kT8")
    # Interleaved steps: each emits one qk psum-pair (PE; evacuated+quantized
    # on DVE) plus two v chunks (PE; evacuated on ACT). v matmuls keep the
    # same moving operand across consecutive MMs so weight loads pipeline.
    v_sb = [None] * N_SKCH
    qk_steps = [
        (jp, a, wi)
        for jp in ((0, 1), (2, 3))
        for a in range(2)
        for wi in range(2)
    ]
    for s, (jp, a, wi) in enumerate(qk_steps):
        W_sb, b_sb, dst = (
            (Wq_sb, bq_sb[a], qT8),
            (Wk_sb, bk_sb[a], kT8),
        )[wi]
        pss = [
            psum.tile([128, 512], f32, name=f"pp{s}_{j}", tag="sc", bufs=2)
            for j in jp
        ]
        for e in range(2):
            for i, j in enumerate(jp):
                nc.tensor.matmul(
                    pss[i][:, :512],
                    lhsT=W_sb[e][:, bass.ts(a, 128)],
                    rhs=xT_sb[e][:, bass.ts(j, SQBLK)],
                    start=(e == 0),
                    stop=(e == 1),
                )
        cpair = (2 * s, 2 * s + 1)
        vps = [
            psum.tile([128, 512], f32, name=f"vp{c}", tag="ps", bufs=4)
            for c in cpair
        ]
        for e in range(2):
            for i, c in enumerate(cpair):
                nc.tensor.matmul(
                    vps[i][:, : A + 2],
                    lhsT=xT_sb[e][:, c * 128 : (c + 1) * 128],
                    rhs=Wv_sb[e],
                    start=(e == 0),
                    stop=False,
                )
        for i, c in enumerate(cpair):
            nc.tensor.matmul(
                vps[i][:, : A + 2],
                lhsT=ones_sb,
                rhs=bv_sb,
                start=False,
                stop=True,
            )
        for i, j in enumerate(jp):
            nc.vector.tensor_scalar_add(
                dst[:, a, bass.ts(j, SQBLK)], pss[i][:, :512], b_sb
            )
        for i, c in enumerate(cpair):
            vt = big.tile([128, A + 2], f16, tag=f"v{c}", name=f"v{c}")
            nc.scalar.copy(vt, vps[i][:, : A + 2])
            v_sb[c] = vt

    # ---- attention ----
    def emit_scores(j, g):
        sc = psum.tile([128, GRP * SQBLK], f32, tag="sc", bufs=2, name=f"sc{j}_{g}")
        for c in range(GRP):
            ch = g * GRP + c
            for sqh in range(2):  # sq 256-col halves (DR moving limit)
                nc.tensor.matmul(
                    sc[:, c * SQBLK + sqh * 256 : c * SQBLK + (sqh + 1) * 256],
                    lhsT=kT8[:, :, ch * 128 : (ch + 1) * 128],
                    rhs=qT8[:, :, j * SQBLK + sqh * 256 : j * SQBLK + (sqh + 1) * 256],
                    start=True,
                    stop=True,
                    perf_mode=DR,
                )
        ex = epool.tile([128, GRP * SQBLK], f16, tag="ex", name=f"ex{j}_{g}")
        nc.scalar.activation(ex, sc, AF.Exp, bias=0.0, scale=float(SCALE))
        at = apool.tile([128, GRP, SQBLK], f16, tag="at", name=f"at{j}_{g}")
        exv = ex.rearrange("p (c s) -> p c s", c=GRP)
        for c in range(GRP):  # per-chunk mask mul: PV c0 needn't wait for c1
            nc.vector.tensor_mul(
                at[:, c, :],
                exv[:, c, :],
                mask_sb[j][:, g * GRP + c, :],
            )
        return at

    def emit_pv(out_ps, at, g):
        for c in range(GRP):
            ch = g * GRP + c
            for sq in range(N_SQSUB):
                nc.tensor.matmul(
                    out_ps[sq][:, : A + 2],
                    lhsT=at[:, c, bass.ts(sq, SQSUB)],
                    rhs=v_sb[ch],
                    start=(ch == 0),
                    stop=(ch == N_SKCH - 1),
                )

    for j in range(N_SQBLK):
        out_ps = [
            psum.tile([128, 512], f32, name=f"out_ps{j}_{s}", tag="ps", bufs=4)
            for s in range(N_SQSUB)
        ]
        ats = {}
        for g in range(N_GRP + PVLAG):
            if g < N_GRP:
                ats[g] = emit_scores(j, g)
            if g >= PVLAG:
                emit_pv(out_ps, ats.pop(g - PVLAG), g - PVLAG)

        ob = opool.tile([128, N_SQSUB, A], f16, tag="ob")
        for sq in range(N_SQSUB):
            rec = spool.tile([128, 1], f32, tag="rec")
            nc.vector.reciprocal(rec, out_ps[sq][:, A : A + 1])
            if sq % 2 == 0:
                nc.vector.tensor_scalar_mul(ob[:, sq, :], out_ps[sq][:, :A], rec)
            else:
                nc.scalar.mul(ob[:, sq, :], out_ps[sq][:, :A], rec)
        nc.sync.dma_start(out=out[j], in_=ob)


def build_nc():
    from contextlib import ExitStack

    import concourse.bacc as bacc
    import concourse.tile as tile
    import concourse.mybir as mybir

    f32 = mybir.dt.float32
    f16 = mybir.dt.float16

    nc = bacc.Bacc("TRN2", target_bir_lowering=False, debug=False)
    xT = nc.dram_tensor("xT", [2, 128, S], f16, kind="ExternalInput").ap()
    maskT = nc.dram_tensor(
        "maskT", [N_SQBLK, 128, N_SKCH, SQBLK], f16, kind="ExternalInput"
    ).ap()
    wpack = nc.dram_tensor("wpack", [128, WPACK_F], f16, kind="ExternalInput").ap()
    bias_pack = nc.dram_tensor("bias_pack", [128, 4], f32, kind="ExternalInput").ap()
    # out[j, p, q, a] = attention_out[j*512 + q*128 + p, a], fp16
    out = nc.dram_tensor(
        "out", [N_SQBLK, 128, N_SQSUB, A], f16, kind="ExternalOutput"
    ).ap()

    tensors = (xT, maskT, wpack, bias_pack, out)
    with tile.TileContext(nc) as tc:
        with ExitStack() as ctx:
            _emit(nc, tc, ctx, tensors)
    nc.compile()
    return nc


def pack_inputs(x, mask, Wq, bq, Wk, bk, Wv, bv):
    """Host-side packing: per-core input maps (core c <- batch c)."""
    hdt = np.float16
    x = np.asarray(x, dtype=np.float32)
    mask = np.asarray(mask)
    # maskT[b, j, p, c, s] = mask[b, j*512+s, c*128+p], as {0.0, 1.0}
    from concurrent.futures import ThreadPoolExecutor

    def _pack_mask(b):
        return np.ascontiguousarray(
            mask[b]
            .transpose(1, 0)
            .reshape(N_SKCH, 128, N_SQBLK, SQBLK)
            .transpose(2, 1, 0, 3)
            .astype(hdt)
        )

    with ThreadPoolExecutor(max_workers=8) as tp:
        mt = list(tp.map(_pack_mask, range(B)))

    wpk = np.zeros((128, WPACK_F), hdt)
    wpk[:, WQ_OFF : WQ_OFF + 2 * A] = (
        np.asarray(Wq, hdt).reshape(2, 128, A).transpose(1, 0, 2).reshape(128, 2 * A)
    )
    wpk[:, WK_OFF : WK_OFF + 2 * A] = (
        np.asarray(Wk, hdt).reshape(2, 128, A).transpose(1, 0, 2).reshape(128, 2 * A)
    )
    Wvp = np.concatenate([np.asarray(Wv, hdt), np.zeros((E, 2), hdt)], axis=1)
    wpk[:, WV_OFF : WV_OFF + 2 * (A + 2)] = (
        Wvp.reshape(2, 128, A + 2).transpose(1, 0, 2).reshape(128, 2 * (A + 2))
    )
    wpk[0, ROW_OFF : ROW_OFF + A] = np.asarray(bv, hdt)
    wpk[0, ROW_OFF + A : ROW_OFF + A + 2] = 1.0
    wpk[0, ROW_OFF + A + 2 : ROW_OFF + A + 2 + 128] = 1.0

    bq = np.asarray(bq, np.float32).reshape(2, 128)
    bk = np.asarray(bk, np.float32).reshape(2, 128)
    bias_pack = np.ascontiguousarray(
        np.stack([bq[0], bq[1], bk[0], bk[1]], axis=1)
    )
    in_maps = []
    for b in range(N_CORES):
        xb = np.ascontiguousarray(x[b].T.astype(hdt)).reshape(2, 128, S)
        in_maps.append(
            {
                "xT": xb,
                "maskT": mt[b],
                "wpack": wpk,
                "bias_pack": bias_pack,
            }
        )
    return in_maps


_NC_CACHE = None


def _get_nc():
    global _NC_CACHE
    if _NC_CACHE is None:
        _NC_CACHE = build_nc()
    return _NC_CACHE


def kernel(x, mask, Wq, bq, Wk, bk, Wv, bv):
    from concourse.bass_utils import run_bass_kernel_spmd

    in_maps = pack_inputs(x, mask, Wq, bq, Wk, bk, Wv, bv)
    nc = _get_nc()
    res = run_bass_kernel_spmd(nc, in_maps, core_ids=list(range(N_CORES)))
    # out[j, p, q, a] -> [j*512 + q*128 + p, a]
    outs = []
    for c in range(N_CORES):
        o = np.asarray(res.results[c]["out"])
        outs.append(o.transpose(0, 2, 1, 3).reshape(S, A))
    return np.stack(outs, axis=0).astype(np.float32)


if __name__ == "__main__":
    nc = build_nc()
    n = sum(len(bb.instructions) for bb in nc.main_func.blocks)
    print("built ok; instructions:", n)


# revision 9
# speedup vs baseline: 1.1761x; 1.1134x over previous
"""Masked attention for (B=8, S=2048, E=A=256), f32 in/out.

Sharding: data-parallel over batch B across the 8 NeuronCores (one batch
element per core, no collectives).

Per-core dataflow (all on-chip after the input DMAs):
  xT[E,S] -> qT8,kT8 ([128, 2, S] fp8e4, a-dim split in 2 k-tiles; bias
             added during the DVE psum evacuation that also quantizes)
          -> v [S, A+2] fp16 (bias via K=1 ones-row matmul; cols A,A+1 are
             1.0 so the PV matmul also produces the softmax denominator)
  scores computed TRANSPOSED via fp8 DoubleRow matmuls (full K=256 in one
  matmul, 2x PE rate): scT[sk chunk=128p, sq 256-sub] into a 2-chunk
  [128, 1024] psum tile (2 banks, double buffered).
  One exp per 2-chunk tile on ACT (scale=1/16, psum->fp16 sbuf); mask
  multiply per single chunk on DVE (fp16 2x mode) to cut the PV dep chain.
  PV is emitted TWO groups behind scores so the exp+mask latency hides
  under ~1.8us of independent PE work.
  outP[sq=128p, A+2] += attnT_chunk.T @ v_chunk  (fp16 PV, accumulate sk)
  out = outP[:, :A] * (1 / outP[:, A])  (DVE recip; per-partition muls
  split DVE/ACT to shorten the j-boundary bubble)

Input DMAs (no SWDGE): sync HWDGE carries one packed weight tensor, two
whole xT[e] transfers, masks j1/j3, and the per-j fp16 output stores;
scalar HWDGE carries masks j0/j2.
"""

import sys

sys.path.insert(0, "/opt/trn_rl_repo")

import numpy as np

B, S, E, A = 8, 2048, 256, 256
N_CORES = 8

SQBLK = 512                 # Sq rows per outer block
N_SQBLK = S // SQBLK        # 4
SQSUB = 128                 # Sq rows per PV psum tile
N_SQSUB = SQBLK // SQSUB    # 4
SKCH = 128                  # Sk rows per score chunk
N_SKCH = S // SKCH          # 16
GRP = 2                     # sk chunks per scores psum tile ([128, GRP*512])
N_GRP = N_SKCH // GRP       # 8
PVLAG = 2                   # groups PV trails scores in the PE stream

SCALE = 1.0 / np.sqrt(np.float32(A))

# wpack column layout (fp16): Wq e0|e1, Wk e0|e1, Wv e0|e1 (A+2 wide),
# then on partition 0 only: bv row (A+2) + ones row (128)
WQ_OFF = 0
WK_OFF = WQ_OFF + 2 * A
WV_OFF = WK_OFF + 2 * A
ROW_OFF = WV_OFF + 2 * (A + 2)
WPACK_F = ROW_OFF + (A + 2) + 128


def _emit(nc, tc, ctx, tensors):
    import concourse.bass as bass
    import concourse.mybir as mybir

    f32 = mybir.dt.float32
    f16 = mybir.dt.float16
    f8 = mybir.dt.float8e4
    AF = mybir.ActivationFunctionType
    DR = mybir.MatmulPerfMode.DoubleRow

    xT, maskT, wpack, bias_pack, out = tensors

    consts = ctx.enter_context(tc.tile_pool(name="consts", bufs=1))
    big = ctx.enter_context(tc.tile_pool(name="big", bufs=1))
    mpool = ctx.enter_context(tc.tile_pool(name="mask", bufs=4))
    epool = ctx.enter_context(tc.tile_pool(name="exp", bufs=4))
    apool = ctx.enter_context(tc.tile_pool(name="attn", bufs=6))
    opool = ctx.enter_context(tc.tile_pool(name="outsb", bufs=2))
    spool = ctx.enter_context(tc.tile_pool(name="small", bufs=8))
    psum = ctx.enter_context(tc.tile_pool(name="psum", bufs=1, space="PSUM"))

    # ---- ACT exp-table preload + PE HAM warm-up during the DMA head ----
    warm_sb = consts.tile([128, 512], f16, tag="warm_sb")
    nc.vector.memset(warm_sb, 1.0)
    warm_ex = consts.tile([128, 1], f16, tag="warm_ex")
    nc.scalar.activation(warm_ex, warm_sb[:, 0:1], AF.Exp, bias=0.0, scale=0.001)
    warm_ps = psum.tile([128, GRP * SQBLK], f32, name="warm_ps", tag="sc", bufs=2)
    for _ in range(10):
        nc.tensor.matmul(
            warm_ps[:, :512], lhsT=warm_sb[:, :128], rhs=warm_sb, start=True, stop=True
        )

    # ---- input DMAs ----
    wp = consts.tile([128, WPACK_F], f16, tag="wpack")
    nc.sync.dma_start(out=wp, in_=wpack)
    bias_sb = consts.tile([128, 4], f32, tag="bias_pack")
    nc.sync.dma_start(out=bias_sb, in_=bias_pack)
    Wq_sb = [wp[:, WQ_OFF + e * A : WQ_OFF + (e + 1) * A] for e in range(2)]
    Wk_sb = [wp[:, WK_OFF + e * A : WK_OFF + (e + 1) * A] for e in range(2)]
    Wv_sb = [
        wp[:, WV_OFF + e * (A + 2) : WV_OFF + (e + 1) * (A + 2)] for e in range(2)
    ]
    bq_sb = [bias_sb[:, 0:1], bias_sb[:, 1:2]]
    bk_sb = [bias_sb[:, 2:3], bias_sb[:, 3:4]]
    bv_sb = wp[0:1, ROW_OFF : ROW_OFF + A + 2]
    ones_sb = wp[0:1, ROW_OFF + A + 2 : ROW_OFF + A + 2 + 128]

    # x per (e, j) so projections start on partial data
    xT_sb = [[None] * N_SQBLK, [None] * N_SQBLK]
    for j in range(N_SQBLK):
        for e in range(2):
            t = big.tile([128, SQBLK], f16, name=f"xt{e}_{j}", tag=f"xT{e}_{j}")
            nc.sync.dma_start(out=t, in_=xT[e][:, bass.ts(j, SQBLK)])
            xT_sb[e][j] = t

    # masks: one 2.1MB DMA per j-block; j0/j2 on the scalar HWDGE ring,
    # j1/j3 on the sync ring (queued behind the small input transfers).
    mask_sb = [None] * N_SQBLK
    for j in (0, 2, 1, 3):
        mt = mpool.tile([128, N_SKCH, SQBLK], f16, name=f"mask{j}", tag="mask")
        (nc.scalar if j % 2 == 0 else nc.sync).dma_start(out=mt, in_=maskT[j])
        mask_sb[j] = mt

    # ---- projections ----
    # qT8: [128, 2, S] fp8e4; [p, t, s] = q[s, t*128+p].
    # kT8 interleaved per chunk: [128, ch, t, skin] so the DR lhsT slice
    # [:, ch, :, :] is CONTIGUOUS (faster LDWEIGHTS than a strided pair).
    qT8 = big.tile([128, 2, S], f8, tag="qT8")
    kT8 = big.tile([128, N_SKCH, 2, SKCH], f8, tag="kT8")
    # Interleaved steps: each emits one qk psum-pair (PE; evacuated+quantized
    # on DVE) plus two v chunks (PE; evacuated on ACT). v matmuls keep the
    # same moving operand across consecutive MMs so weight loads pipeline.
    v_sb = [None] * N_SKCH
    qk_steps = [
        (jp, a, wi)
        for jp in ((0, 1), (2, 3))
        for a in range(2)
        for wi in range(2)
    ]
    for s, (jp, a, wi) in enumerate(qk_steps):
        W_sb, b_sb = ((Wq_sb, bq_sb[a]), (Wk_sb, bk_sb[a]))[wi]
        pss = [
            psum.tile([128, 512], f32, name=f"pp{s}_{j}", tag="sc", bufs=2)
            for j in jp
        ]
        for e in range(2):
            for i, j in enumerate(jp):
                nc.tensor.matmul(
                    pss[i][:, :512],
                    lhsT=W_sb[e][:, bass.ts(a, 128)],
                    rhs=xT_sb[e][j],
                    start=(e == 0),
                    stop=(e == 1),
                )
        cpair = (2 * s, 2 * s + 1)
        vps = [
            psum.tile([128, 512], f32, name=f"vp{c}", tag="ps", bufs=4)
            for c in cpair
        ]
        for e in range(2):
            for i, c in enumerate(cpair):
                nc.tensor.matmul(
                    vps[i][:, : A + 2],
                    lhsT=xT_sb[e][c // 4][:, bass.ts(c % 4, 128)],
                    rhs=Wv_sb[e],
                    start=(e == 0),
                    stop=False,
                )
        for i, c in enumerate(cpair):
            nc.tensor.matmul(
                vps[i][:, : A + 2],
                lhsT=ones_sb,
                rhs=bv_sb,
                start=False,
                stop=True,
            )
        for i, j in enumerate(jp):
            if wi == 0:
                dst = qT8[:, a, bass.ts(j, SQBLK)]
                src = pss[i][:, :512]
            else:
                dst = kT8[:, 4 * j : 4 * (j + 1), a, :]
                src = pss[i][:, :512].rearrange("p (c s) -> p c s", c=4)
            nc.vector.tensor_scalar_add(dst, src, b_sb)
        for i, c in enumerate(cpair):
            vt = big.tile([128, A + 2], f16, tag=f"v{c}", name=f"v{c}")
            nc.scalar.copy(vt, vps[i][:, : A + 2])
            v_sb[c] = vt

    # ---- attention: flat software pipeline over all (j, g) groups ----
    # PV trails scores by PVLAG groups (incl. across j boundaries) so the
    # exp+mask chain latency hides under independent PE work.
    def emit_scores(j, g):
        sc = psum.tile([128, GRP * SQBLK], f32, tag="sc", bufs=2, name=f"sc{j}_{g}")
        for c in range(GRP):
            ch = g * GRP + c
            for sqh in range(2):  # sq 256-col halves (DR moving limit)
                nc.tensor.matmul(
                    sc[:, c * SQBLK + sqh * 256 : c * SQBLK + (sqh + 1) * 256],
                    lhsT=kT8[:, ch, :, :],
                    rhs=qT8[:, :, j * SQBLK + sqh * 256 : j * SQBLK + (sqh + 1) * 256],
                    start=True,
                    stop=True,
                    perf_mode=DR,
                )
        ex = epool.tile([128, GRP * SQBLK], f16, tag="ex", name=f"ex{j}_{g}")
        nc.scalar.activation(ex, sc, AF.Exp, bias=0.0, scale=float(SCALE))
        at = apool.tile([128, GRP, SQBLK], f16, tag="at", name=f"at{j}_{g}")
        exv = ex.rearrange("p (c s) -> p c s", c=GRP)
        for c in range(GRP):  # per-chunk mask mul: PV c0 needn't wait for c1
            nc.vector.tensor_mul(
                at[:, c, :],
                exv[:, c, :],
                mask_sb[j][:, g * GRP + c, :],
            )
        return at

    def emit_pv(out_ps, at, g):
        for c in range(GRP):
            ch = g * GRP + c
            for sq in range(N_SQSUB):
                nc.tensor.matmul(
                    out_ps[sq][:, : A + 2],
                    lhsT=at[:, c, bass.ts(sq, SQSUB)],
                    rhs=v_sb[ch],
                    start=(ch == 0),
                    stop=(ch == N_SKCH - 1),
                )

    NG_ALL = N_SQBLK * N_GRP
    ats = {}
    out_ps = None
    for G in range(NG_ALL + PVLAG):
        if G < NG_ALL:
            ats[G] = emit_scores(G // N_GRP, G % N_GRP)
        Gp = G - PVLAG
        if Gp >= 0:
            jP, gP = Gp // N_GRP, Gp % N_GRP
            if gP == 0:
                out_ps = [
                    psum.tile(
                        [128, 512], f32, name=f"out_ps{jP}_{s}", tag="ps", bufs=4
                    )
                    for s in range(N_SQSUB)
                ]
            emit_pv(out_ps, ats.pop(Gp), gP)
            if gP == N_GRP - 1:
                ob = opool.tile([128, N_SQSUB, A], f16, tag="ob", name=f"ob{jP}")
                for sq in range(N_SQSUB):
                    rec = spool.tile([128, 1], f32, tag="rec", name=f"rec{jP}_{sq}")
                    nc.vector.reciprocal(rec, out_ps[sq][:, A : A + 1])
                    if sq % 2 == 0:
                        nc.vector.tensor_scalar_mul(
                            ob[:, sq, :], out_ps[sq][:, :A], rec
                        )
                    else:
                        nc.scalar.mul(ob[:, sq, :], out_ps[sq][:, :A], rec)
                nc.sync.dma_start(out=out[jP], in_=ob)


def build_nc():
    from contextlib import ExitStack

    import concourse.bacc as bacc
    import concourse.tile as tile
    import concourse.mybir as mybir

    f32 = mybir.dt.float32
    f16 = mybir.dt.float16

    nc = bacc.Bacc("TRN2", target_bir_lowering=False, debug=False)
    xT = nc.dram_tensor("xT", [2, 128, S], f16, kind="ExternalInput").ap()
    maskT = nc.dram_tensor(
        "maskT", [N_SQBLK, 128, N_SKCH, SQBLK], f16, kind="ExternalInput"
    ).ap()
    wpack = nc.dram_tensor("wpack", [128, WPACK_F], f16, kind="ExternalInput").ap()
    bias_pack = nc.dram_tensor("bias_pack", [128, 4], f32, kind="ExternalInput").ap()
    # out[j, p, q, a] = attention_out[j*512 + q*128 + p, a], fp16
    out = nc.dram_tensor(
        "out", [N_SQBLK, 128, N_SQSUB, A], f16, kind="ExternalOutput"
    ).ap()

    tensors = (xT, maskT, wpack, bias_pack, out)
    with tile.TileContext(nc) as tc:
        with ExitStack() as ctx:
            _emit(nc, tc, ctx, tensors)
    nc.compile()
    return nc


def pack_inputs(x, mask, Wq, bq, Wk, bk, Wv, bv):
    """Host-side packing: per-core input maps (core c <- batch c)."""
    hdt = np.float16
    x = np.asarray(x, dtype=np.float32)
    mask = np.asarray(mask)
    # maskT[b, j, p, c, s] = mask[b, j*512+s, c*128+p], as {0.0, 1.0}
    from concurrent.futures import ThreadPoolExecutor

    def _pack_mask(b):
        return np.ascontiguousarray(
            mask[b]
            .transpose(1, 0)
            .reshape(N_SKCH, 128, N_SQBLK, SQBLK)
            .transpose(2, 1, 0, 3)
            .astype(hdt)
        )

    with ThreadPoolExecutor(max_workers=8) as tp:
        mt = list(tp.map(_pack_mask, range(B)))

    wpk = np.zeros((128, WPACK_F), hdt)
    wpk[:, WQ_OFF : WQ_OFF + 2 * A] = (
        np.asarray(Wq, hdt).reshape(2, 128, A).transpose(1, 0, 2).reshape(128, 2 * A)
    )
    wpk[:, WK_OFF : WK_OFF + 2 * A] = (
        np.asarray(Wk, hdt).reshape(2, 128, A).transpose(1, 0, 2).reshape(128, 2 * A)
    )
    Wvp = np.concatenate([np.asarray(Wv, hdt), np.zeros((E, 2), hdt)], axis=1)
    wpk[:, WV_OFF : WV_OFF + 2 * (A + 2)] = (
        Wvp.reshape(2, 128, A + 2).transpose(1, 0, 2).reshape(128, 2 * (A + 2))
    )
    wpk[0, ROW_OFF : ROW_OFF + A] = np.asarray(bv, hdt)
    wpk[0, ROW_OFF + A : ROW_OFF + A + 2] = 1.0
    wpk[0, ROW_OFF + A + 2 : ROW_OFF + A + 2 + 128] = 1.0

    bq = np.asarray(bq, np.float32).reshape(2, 128)
    bk = np.asarray(bk, np.float32).reshape(2, 128)
    bias_pack = np.ascontiguousarray(
        np.stack([bq[0], bq[1], bk[0], bk[1]], axis=1)
    )
    in_maps = []
    for b in range(N_CORES):
        xb = np.ascontiguousarray(x[b].T.astype(hdt)).reshape(2, 128, S)
        in_maps.append(
            {
                "xT": xb,
                "maskT": mt[b],
                "wpack": wpk,
                "bias_pack": bias_pack,
            }
        )
    return in_maps


_NC_CACHE = None


def _get_nc():
    global _NC_CACHE
    if _NC_CACHE is None:
        _NC_CACHE = build_nc()
    return _NC_CACHE


def kernel(x, mask, Wq, bq, Wk, bk, Wv, bv):
    from concourse.bass_utils import run_bass_kernel_spmd

    in_maps = pack_inputs(x, mask, Wq, bq, Wk, bk, Wv, bv)
    nc = _get_nc()
    res = run_bass_kernel_spmd(nc, in_maps, core_ids=list(range(N_CORES)))
    # out[j, p, q, a] -> [j*512 + q*128 + p, a]
    outs = []
    for c in range(N_CORES):
        o = np.asarray(res.results[c]["out"])
        outs.append(o.transpose(0, 2, 1, 3).reshape(S, A))
    return np.stack(outs, axis=0).astype(np.float32)


if __name__ == "__main__":
    nc = build_nc()
    n = sum(len(bb.instructions) for bb in nc.main_func.blocks)
    print("built ok; instructions:", n)
